# revision 27
# baseline (speedup 1.0000x reference)
"""BottleNeck-MHSA (B=16, C=512, H=W=32, NH=8, DK=64) on 8 Trainium2 cores.

Sharding: pure data-parallel over batch (2 batches per core), no collectives.

Kernel design (per core), v2:
- Weights pre-permuted host-side to head-major channel order c' = nh*64 + d.
- Rel-pos bias folded into the energy matmul via an augmented K=128
  contraction: Qaug rows = [qT(64) | ahT(32) | awT(32)], Kaug rows =
  [kT(64) | OneHotKx(32) | OneHotKy(32)].
- ahT/awT computed DIRECTLY with small block-diagonal matmuls against
  host-prepared rel_h.T / rel_w.T slices (no DRAM round trip, no gather
  DMAs): per qx-block one K=128 M=64 N=32 bf16 matmul yields both heads'
  ahT rows; awT computed qy-major (strided rhs) and permuted during the
  PSUM->SBUF copy.
- Onehot rows of Kaug DMA'd once at startup straight from DRAM.
- Softmax skips max-subtraction; exp on ACT with 1/sqrt(DK) fused;
  normalization deferred past AV: lhsT = [V | ones*64] so denominators
  come out in rows 64-127 of the AV PSUM; reciprocal_approx_fast on DVE.
- exp scores and V in bf16 (fp32 PSUM accumulation), everything else fp32.
- x tiles double-buffered so batch b+1 DMA overlaps batch b attention.
"""

from contextlib import ExitStack

import numpy as np

import concourse.bass as bass
import concourse.tile as tile
from concourse import bacc, mybir
from concourse.ap import AP
from concourse.bass_utils import run_bass_kernel_spmd

FP32 = mybir.dt.float32
FP32R = mybir.dt.float32r
BF16 = mybir.dt.bfloat16
Exp = mybir.ActivationFunctionType.Exp

B = 16
C = 512
N = 1024
NH = 8
DK = 64
HW = 32
NCORES = 8
NB = B // NCORES  # batches per core


def _build_body(ctx: ExitStack, tc: tile.TileContext, outs, ins, NB: int):
    nc = tc.nc
    x_in, wq_in, wk_in, wv_in, wo_in, oh_in, rh_in, rw_in, bo_in = ins
    y_out = outs[0]

    consts = ctx.enter_context(tc.tile_pool(name="consts", bufs=1))
    persist = ctx.enter_context(tc.tile_pool(name="persist", bufs=1))
    xpool = ctx.enter_context(tc.tile_pool(name="xpool", bufs=2))
    work = ctx.enter_context(tc.tile_pool(name="work", bufs=2))
    expp = ctx.enter_context(tc.tile_pool(name="expp", bufs=9))
    psum = ctx.enter_context(tc.tile_pool(name="psum", bufs=3, space="PSUM"))
    psum_av = ctx.enter_context(tc.tile_pool(name="psum_av", bufs=1, space="PSUM"))

    # ---------------- constants (wq first: first matmul needs it) ----------------
    w_t = {}
    for nm, src in (("wq", wq_in), ("wk", wk_in), ("wv", wv_in), ("wo", wo_in)):
        for kc in range(4):
            t = consts.tile([128, C], FP32R, tag=f"{nm}{kc}", name=f"{nm}{kc}")
            w_t[nm, kc] = t
    for kc in range(4):
        nc.sync.dma_start(w_t["wq", kc][:], wq_in[kc * 128 : (kc + 1) * 128, :])
    rh_t = consts.tile([128, 2048], BF16, tag="rht", name="rht")
    nc.sync.dma_start(rh_t[:], rh_in[:])
    rw_t = consts.tile([128, 2048], BF16, tag="rwt", name="rwt")
    nc.sync.dma_start(rw_t[:], rw_in[:])
    bo_t = consts.tile([128, 4], FP32, tag="bo", name="bo")
    nc.sync.dma_start(bo_t[:], bo_in[:].rearrange("(c p) one -> p (c one)", p=128))

    # ---------------- persistent work tiles ----------------
    qaug = [persist.tile([128, N], FP32R, tag=f"qaug{h}", name=f"qaug{h}") for h in range(NH)]
    kaug = [persist.tile([128, N], FP32R, tag=f"kaug{h}", name=f"kaug{h}") for h in range(NH)]
    vaug = [
        [persist.tile([128, 128], BF16, tag=f"vaug{h}_{jb}", name=f"vaug{h}_{jb}") for jb in range(8)]
        for h in range(NH)
    ]
    oin = [qaug[2 * kc] for kc in range(4)]  # reuse: qaug[2kc] dead after S^T of head 2kc

    # onehot rows of kaug are constant across batches: DMA once
    for h in range(NH):
        nc.sync.dma_start(kaug[h][64:128, :], oh_in[:])
    for h in range(NH):
        for jb in range(8):
            nc.vector.memset(vaug[h][jb][:, 64:128], 1.0)

    def x_tiles(b):
        ts = [xpool.tile([128, N], FP32R, tag=f"x{kc}", name=f"x{kc}_b{b}") for kc in range(4)]
        for kc in range(4):
            for nn in range(2):
                nc.sync.dma_start(
                    ts[kc][:, nn * 512 : (nn + 1) * 512],
                    x_in[b, kc * 128 : (kc + 1) * 128, nn * 512 : (nn + 1) * 512],
                )
        return ts

    xt_cur = x_tiles(0)

    for b in range(NB):
        xt = xt_cur
        if b == 0:
            # non-critical weight loads after the first x chunks
            for nm, src_ap in (("wk", wk_in), ("wv", wv_in), ("wo", wo_in)):
                for kc in range(4):
                    nc.sync.dma_start(w_t[nm, kc][:], src_ap[kc * 128 : (kc + 1) * 128, :])
        if b + 1 < NB:
            xt_cur = x_tiles(b + 1)

        for mc in range(4):
            hA, hB = 2 * mc, 2 * mc + 1
            # ---- Q projection ----
            pq = psum.tile([128, N], FP32, tag="mm", name="mm")
            for kc in range(4):
                for nn in range(2):
                    nc.tensor.matmul(
                        pq[:, nn * 512 : (nn + 1) * 512],
                        w_t["wq", kc][:, mc * 128 : (mc + 1) * 128],
                        xt[kc][:, nn * 512 : (nn + 1) * 512],
                        start=(kc == 0),
                        stop=(kc == 3),
                    )
            # q -> qaug rows 0:64 (full precision, straight from PSUM)
            nc.scalar.copy(qaug[hA][0:64, :], pq[0:64, :])
            nc.vector.tensor_copy(qaug[hB][0:64, :], pq[64:128, :])
            # bf16 copy of q for the rel-bias matmuls
            qpair = work.tile([128, N], BF16, tag="qpair", name="qpair", bufs=2)
            nc.vector.tensor_copy(qpair[:, 0:512], pq[:, 0:512])
            nc.vector.tensor_copy(qpair[:, 512:1024], pq[:, 512:1024])

            # ---- K projection ----
            pk = psum.tile([128, N], FP32, tag="mm", name="mm")
            for kc in range(4):
                for nn in range(2):
                    nc.tensor.matmul(
                        pk[:, nn * 512 : (nn + 1) * 512],
                        w_t["wk", kc][:, mc * 128 : (mc + 1) * 128],
                        xt[kc][:, nn * 512 : (nn + 1) * 512],
                        start=(kc == 0),
                        stop=(kc == 3),
                    )
            nc.scalar.copy(kaug[hA][0:64, :], pk[0:64, :])
            nc.vector.tensor_copy(kaug[hB][0:64, :], pk[64:128, :])

            # ---- rel-pos bias: ahT/awT via block-diag K=128 matmuls ----
            # pbh rows 0:32 = ahA, 32:64 = ahB (qp-major)
            # pbw rows 0:32 = awA, 32:64 = awB (qy-major; copy permutes)
            pbh = psum.tile([64, N], FP32, tag="mm", name="mm")
            pbw = psum.tile([64, N], FP32, tag="mm", name="mm")
            qp_ap = qpair[:]
            for i in range(32):
                qx = i
                blk = slice(qx * 32, (qx + 1) * 32)
                nc.tensor.matmul(
                    pbh[0:64, blk],
                    rh_t[:, qx * 64 : (qx + 1) * 64],
                    qpair[:, blk],
                    start=True, stop=True,
                )
                qy = i
                rhs = AP(qp_ap.tensor, qp_ap.offset + qy, [[N, 128], [32, 32]])
                nc.tensor.matmul(
                    pbw[0:64, qy * 32 : (qy + 1) * 32],
                    rw_t[:, qy * 64 : (qy + 1) * 64],
                    rhs,
                    start=True, stop=True,
                )
            nc.vector.tensor_copy(qaug[hA][64:96, :], pbh[0:32, :])
            nc.scalar.copy(qaug[hB][64:96, :], pbh[32:64, :])
            for hh, h in ((0, hA), (1, hB)):
                pw = pbw[hh * 32 : (hh + 1) * 32, :]
                src = AP(pw.tensor, pw.offset, [[N, 32], [1, 32], [32, 32]])
                nc.vector.tensor_copy(
                    qaug[h][96:128, :].rearrange("p (a b) -> p a b", a=32), src
                )

            # ---- V projection ----
            for nb in (2 * mc, 2 * mc + 1):
                pv = psum.tile([128, 512], FP32, tag="mm", name="mm")
                for kc in range(4):
                    nc.tensor.matmul(
                        pv[:],
                        xt[kc][:, nb * 128 : (nb + 1) * 128],
                        w_t["wv", kc][:],
                        start=(kc == 0),
                        stop=(kc == 3),
                    )
                for h in range(NH):
                    nc.vector.tensor_copy(vaug[h][nb][:, 0:64], pv[:, h * 64 : h * 64 + 64])

        # ---- attention: S^T/exp of head h interleaved with AV of head h-1 ----
        est_all = [None] * NH

        def emit_st(h):
            est = [expp.tile([128, N], BF16, tag="expst", name="expst") for jb in range(8)]
            est_all[h] = est
            for jb in range(8):
                pst = psum.tile([128, N], FP32, tag="mm", name="mm")
                for nn in range(2):
                    nc.tensor.matmul(
                        pst[:, nn * 512 : (nn + 1) * 512],
                        kaug[h][:, jb * 128 : (jb + 1) * 128],
                        qaug[h][:, nn * 512 : (nn + 1) * 512],
                        start=True,
                        stop=True,
                    )
                nc.scalar.activation(est[jb][:], pst[:], Exp, bias=0.0, scale=0.125)

        def emit_av(h):
            est = est_all[h]
            pav = psum_av.tile([128, N], FP32, tag="av", name="av")
            for jb in range(8):
                for nn in range(2):
                    nc.tensor.matmul(
                        pav[:, nn * 512 : (nn + 1) * 512],
                        vaug[h][jb][:],
                        est[jb][:, nn * 512 : (nn + 1) * 512],
                        start=(jb == 0),
                        stop=(jb == 7),
                    )
            # NB: reciprocal_approx_fast (custom DVE op) breaks on partition-
            # offset sources — stage the denominator at partition base 0 first.
            # Keep the whole normalize chain on DVE so it hides under the
            # next head's (ACT-paced) S^T/exp stream.
            den = work.tile([64, N], FP32, tag="den", name="den", bufs=1)
            nc.vector.tensor_copy(den[:], pav[64:128, :])
            recip = work.tile([64, N], FP32, tag="recip", name="recip", bufs=1)
            nc.vector.reciprocal_approx_fast(recip[:], den[:])
            nc.vector.tensor_mul(
                oin[h // 2][(h % 2) * 64 : (h % 2) * 64 + 64, :], pav[0:64, :], recip[:]
            )

        emit_st(0)
        for h in range(1, NH):
            emit_st(h)
            emit_av(h - 1)
        emit_av(NH - 1)

        # ---------------- O projection + bias ----------------
        for mc in range(4):
            po = psum.tile([128, N], FP32, tag="mm", name="mm")
            for kc in range(4):
                for nn in range(2):
                    nc.tensor.matmul(
                        po[:, nn * 512 : (nn + 1) * 512],
                        w_t["wo", kc][:, mc * 128 : (mc + 1) * 128],
                        oin[kc][:, nn * 512 : (nn + 1) * 512],
                        start=(kc == 0),
                        stop=(kc == 3),
                    )
            oo = work.tile([128, N], FP32, tag="oout", name="oout")
            nc.vector.tensor_add(oo[:], po[:], bo_t[:, mc : mc + 1].broadcast_to((128, N)))
            nc.sync.dma_start(y_out[b, mc * 128 : (mc + 1) * 128, :], oo[:])


def _host_prep(w_q, w_k, w_v, w_o, b_o, rel_h, rel_w):
    import ml_dtypes

    perm = np.array([(c % 64) * 8 + c // 64 for c in range(C)])  # c' -> orig c
    oh = np.zeros((64, N), np.float32)
    j = np.arange(N)
    oh[j // HW, j] = 1.0
    oh[32 + j % HW, j] = 1.0
    # block-diag slices: col block qx holds [headA: rel_h.T[:, 31-qx:63-qx] | 0]
    # stacked with [0 | headB: same slice] so one K=128 M=64 matmul yields
    # both heads' ahT rows.
    rhT = rel_h.T.astype(np.float32)
    rwT = rel_w.T.astype(np.float32)
    rh2 = np.zeros((128, 2048), np.float32)
    rw2 = np.zeros((128, 2048), np.float32)
    for qq in range(32):
        sl = slice(31 - qq, 63 - qq)
        rh2[0:64, qq * 64 : qq * 64 + 32] = rhT[:, sl]
        rh2[64:128, qq * 64 + 32 : qq * 64 + 64] = rhT[:, sl]
        rw2[0:64, qq * 64 : qq * 64 + 32] = rwT[:, sl]
        rw2[64:128, qq * 64 + 32 : qq * 64 + 64] = rwT[:, sl]
    rh2 = rh2.astype(ml_dtypes.bfloat16)
    rw2 = rw2.astype(ml_dtypes.bfloat16)
    return dict(
        wq=np.ascontiguousarray(w_q[perm, :].T, dtype=np.float32),
        wk=np.ascontiguousarray(w_k[perm, :].T, dtype=np.float32),
        wv=np.ascontiguousarray(w_v[perm, :].T, dtype=np.float32),
        wo=np.ascontiguousarray(w_o.T, dtype=np.float32),
        onehot=oh,
        relh2=rh2,
        relw2=rw2,
        bo=np.ascontiguousarray(b_o.reshape(C, 1), dtype=np.float32),
    )


_CACHE = {}


def _build_program():
    if "nc" in _CACHE:
        return _CACHE["nc"], _CACHE["names"]
    nc = bacc.Bacc("TRN2", target_bir_lowering=False, debug=False, num_devices=NCORES)
    specs = [
        ("x", (NB, C, N), FP32R),
        ("wq", (C, C), FP32R),
        ("wk", (C, C), FP32R),
        ("wv", (C, C), FP32R),
        ("wo", (C, C), FP32R),
        ("onehot", (64, N), FP32R),
        ("relh2", (128, 2048), BF16),
        ("relw2", (128, 2048), BF16),
        ("bo", (C, 1), FP32),
    ]
    in_aps = [nc.dram_tensor(nm, list(shape), dt, kind="ExternalInput").ap() for nm, shape, dt in specs]
    out_ap = nc.dram_tensor("y", [NB, C, N], FP32, kind="ExternalOutput").ap()
    with tile.TileContext(nc) as tc:
        with ExitStack() as ctx:
            _build_body(ctx, tc, [out_ap], in_aps, NB)
    nc.compile()
    _CACHE["nc"] = nc
    _CACHE["names"] = [s[0] for s in specs]
    return nc, _CACHE["names"]


def _run(inputs, trace=False, tmpdir=None):
    x = np.asarray(inputs["x"], dtype=np.float32)
    cst = _host_prep(
        np.asarray(inputs["w_q"], np.float32),
        np.asarray(inputs["w_k"], np.float32),
        np.asarray(inputs["w_v"], np.float32),
        np.asarray(inputs["w_o"], np.float32),
        np.asarray(inputs["b_o"], np.float32),
        np.asarray(inputs["rel_h"], np.float32),
        np.asarray(inputs["rel_w"], np.float32),
    )
    nc, _ = _build_program()
    in_maps = []
    for c in range(NCORES):
        m = dict(cst)
        m["x"] = np.ascontiguousarray(x[c * NB : (c + 1) * NB].reshape(NB, C, N))
        in_maps.append(m)
    res = run_bass_kernel_spmd(
        nc, in_maps, core_ids=list(range(NCORES)), trace=trace, tmpdir=tmpdir
    )
    out = np.empty((B, C, HW, HW), np.float32)
    for c in range(NCORES):
        out[c * NB : (c + 1) * NB] = res.results[c]["y"].reshape(NB, C, HW, HW)
    return out, res


def kernel(**inputs):
    out, _ = _run(inputs, trace=False)
    return out


# revision 28
# speedup vs baseline: 1.1277x; 1.1277x over previous
"""BottleNeck-MHSA (B=16, C=512, H=W=32, NH=8, DK=64) on 8 Trainium2 cores.

Sharding: pure data-parallel over batch (2 batches per core), no collectives.

Kernel design (per core), v2:
- Weights pre-permuted host-side to head-major channel order c' = nh*64 + d.
- Rel-pos bias folded into the energy matmul via an augmented K=128
  contraction: Qaug rows = [qT(64) | ahT(32) | awT(32)], Kaug rows =
  [kT(64) | OneHotKx(32) | OneHotKy(32)].
- ahT/awT computed DIRECTLY with small block-diagonal matmuls against
  host-prepared rel_h.T / rel_w.T slices (no DRAM round trip, no gather
  DMAs): per qx-block one K=128 M=64 N=32 bf16 matmul yields both heads'
  ahT rows; awT computed qy-major (strided rhs) and permuted during the
  PSUM->SBUF copy.
- Onehot rows of Kaug DMA'd once at startup straight from DRAM.
- Softmax skips max-subtraction; exp on ACT with 1/sqrt(DK) fused;
  normalization deferred past AV: lhsT = [V | ones*64] so denominators
  come out in rows 64-127 of the AV PSUM; reciprocal_approx_fast on DVE.
- exp scores and V in bf16 (fp32 PSUM accumulation), everything else fp32.
- x tiles double-buffered so batch b+1 DMA overlaps batch b attention.
"""

from contextlib import ExitStack

import numpy as np

import concourse.bass as bass
import concourse.tile as tile
from concourse import bacc, mybir
from concourse.ap import AP
from concourse.bass_utils import run_bass_kernel_spmd

FP32 = mybir.dt.float32
FP32R = mybir.dt.float32r
BF16 = mybir.dt.bfloat16
Exp = mybir.ActivationFunctionType.Exp

B = 16
C = 512
N = 1024
NH = 8
DK = 64
HW = 32
NCORES = 8
NB = B // NCORES  # batches per core


def _build_body(ctx: ExitStack, tc: tile.TileContext, outs, ins, NB: int):
    nc = tc.nc
    x_in, wq_in, wk_in, wv_in, wo_in, oh_in, rh_in, rw_in, bo_in = ins
    y_out = outs[0]

    consts = ctx.enter_context(tc.tile_pool(name="consts", bufs=1))
    persist = ctx.enter_context(tc.tile_pool(name="persist", bufs=1))
    xpool = ctx.enter_context(tc.tile_pool(name="xpool", bufs=2))
    work = ctx.enter_context(tc.tile_pool(name="work", bufs=2))
    expp = ctx.enter_context(tc.tile_pool(name="expp", bufs=9))
    psum = ctx.enter_context(tc.tile_pool(name="psum", bufs=3, space="PSUM"))
    psum_av = ctx.enter_context(tc.tile_pool(name="psum_av", bufs=1, space="PSUM"))

    # ---------------- constants (wq first: first matmul needs it) ----------------
    w_t = {}
    for nm, src in (("wq", wq_in), ("wk", wk_in), ("wv", wv_in), ("wo", wo_in)):
        for kc in range(4):
            t = consts.tile([128, C], FP32R, tag=f"{nm}{kc}", name=f"{nm}{kc}")
            w_t[nm, kc] = t
    rh_t = consts.tile([128, 2048], BF16, tag="rht", name="rht")
    rw_t = consts.tile([128, 2048], BF16, tag="rwt", name="rwt")
    bo_t = consts.tile([128, 4], FP32, tag="bo", name="bo")

    # ---------------- persistent work tiles ----------------
    qaug = [persist.tile([128, N], FP32R, tag=f"qaug{h}", name=f"qaug{h}") for h in range(NH)]
    kaug = [persist.tile([128, N], FP32R, tag=f"kaug{h}", name=f"kaug{h}") for h in range(NH)]
    vaug = [
        [persist.tile([128, 128], BF16, tag=f"vaug{h}_{jb}", name=f"vaug{h}_{jb}") for jb in range(8)]
        for h in range(NH)
    ]
    oin = [qaug[2 * kc] for kc in range(4)]  # reuse: qaug[2kc] dead after S^T of head 2kc

    for h in range(NH):
        for jb in range(8):
            nc.vector.memset(vaug[h][jb][:, 64:128], 1.0)

    def x_tiles(b):
        ts = [xpool.tile([128, N], FP32R, tag=f"x{kc}", name=f"x{kc}_b{b}") for kc in range(4)]
        for kc in range(4):
            for nn in range(2):
                nc.sync.dma_start(
                    ts[kc][:, nn * 512 : (nn + 1) * 512],
                    x_in[b, kc * 128 : (kc + 1) * 128, nn * 512 : (nn + 1) * 512],
                )
        return ts

    # Progressive startup: interleave wq[kc] with x[kc] chunks so the first
    # Q-proj matmul (needs wq0+x0, ~1.5 MB) starts ~4us in, then stream the
    # rest in first-use order: wk (K proj), rel tables (bias), wv, onehot
    # rows of kaug (S^T), wo/bias late.
    xt0 = [xpool.tile([128, N], FP32R, tag=f"x{kc}", name=f"x{kc}_b0") for kc in range(4)]
    for kc in range(4):
        nc.sync.dma_start(w_t["wq", kc][:], wq_in[kc * 128 : (kc + 1) * 128, :])
        for nn in range(2):
            nc.sync.dma_start(
                xt0[kc][:, nn * 512 : (nn + 1) * 512],
                x_in[0, kc * 128 : (kc + 1) * 128, nn * 512 : (nn + 1) * 512],
            )
    for kc in range(4):
        nc.sync.dma_start(w_t["wk", kc][:], wk_in[kc * 128 : (kc + 1) * 128, :])
    nc.sync.dma_start(rh_t[:], rh_in[:])
    nc.sync.dma_start(rw_t[:], rw_in[:])
    for kc in range(4):
        nc.sync.dma_start(w_t["wv", kc][:], wv_in[kc * 128 : (kc + 1) * 128, :])
    # onehot rows of kaug are constant across batches: DMA once
    for h in range(NH):
        nc.sync.dma_start(kaug[h][64:128, :], oh_in[:])
    for kc in range(4):
        nc.sync.dma_start(w_t["wo", kc][:], wo_in[kc * 128 : (kc + 1) * 128, :])
    nc.sync.dma_start(bo_t[:], bo_in[:].rearrange("(c p) one -> p (c one)", p=128))
    xt_cur = xt0

    for b in range(NB):
        xt = xt_cur
        if b + 1 < NB:
            xt_cur = x_tiles(b + 1)

        for mc in range(4):
            hA, hB = 2 * mc, 2 * mc + 1
            # ---- Q projection ----
            pq = psum.tile([128, N], FP32, tag="mm", name="mm")
            for kc in range(4):
                for nn in range(2):
                    nc.tensor.matmul(
                        pq[:, nn * 512 : (nn + 1) * 512],
                        w_t["wq", kc][:, mc * 128 : (mc + 1) * 128],
                        xt[kc][:, nn * 512 : (nn + 1) * 512],
                        start=(kc == 0),
                        stop=(kc == 3),
                    )
            # q -> qaug rows 0:64 (full precision, straight from PSUM)
            nc.vector.tensor_copy(qaug[hA][0:64, :], pq[0:64, :])
            nc.vector.tensor_copy(qaug[hB][0:64, :], pq[64:128, :])
            # bf16 copy of q for the rel-bias matmuls
            qpair = work.tile([128, N], BF16, tag="qpair", name="qpair", bufs=2)
            nc.vector.tensor_copy(qpair[:, 0:512], pq[:, 0:512])
            nc.vector.tensor_copy(qpair[:, 512:1024], pq[:, 512:1024])

            # ---- K projection ----
            pk = psum.tile([128, N], FP32, tag="mm", name="mm")
            for kc in range(4):
                for nn in range(2):
                    nc.tensor.matmul(
                        pk[:, nn * 512 : (nn + 1) * 512],
                        w_t["wk", kc][:, mc * 128 : (mc + 1) * 128],
                        xt[kc][:, nn * 512 : (nn + 1) * 512],
                        start=(kc == 0),
                        stop=(kc == 3),
                    )
            nc.vector.tensor_copy(kaug[hA][0:64, :], pk[0:64, :])
            nc.vector.tensor_copy(kaug[hB][0:64, :], pk[64:128, :])

            # ---- rel-pos bias: ahT/awT via block-diag K=128 matmuls ----
            # pbh rows 0:32 = ahA, 32:64 = ahB (qp-major)
            # pbw rows 0:32 = awA, 32:64 = awB (qy-major; copy permutes)
            pbh = psum.tile([64, N], FP32, tag="mm", name="mm")
            pbw = psum.tile([64, N], FP32, tag="mm", name="mm")
            qp_ap = qpair[:]
            for i in range(32):
                qx = i
                blk = slice(qx * 32, (qx + 1) * 32)
                nc.tensor.matmul(
                    pbh[0:64, blk],
                    rh_t[:, qx * 64 : (qx + 1) * 64],
                    qpair[:, blk],
                    start=True, stop=True,
                )
                qy = i
                rhs = AP(qp_ap.tensor, qp_ap.offset + qy, [[N, 128], [32, 32]])
                nc.tensor.matmul(
                    pbw[0:64, qy * 32 : (qy + 1) * 32],
                    rw_t[:, qy * 64 : (qy + 1) * 64],
                    rhs,
                    start=True, stop=True,
                )
            nc.vector.tensor_copy(qaug[hA][64:96, :], pbh[0:32, :])
            nc.vector.tensor_copy(qaug[hB][64:96, :], pbh[32:64, :])
            for hh, h in ((0, hA), (1, hB)):
                pw = pbw[hh * 32 : (hh + 1) * 32, :]
                src = AP(pw.tensor, pw.offset, [[N, 32], [1, 32], [32, 32]])
                nc.vector.tensor_copy(
                    qaug[h][96:128, :].rearrange("p (a b) -> p a b", a=32), src
                )

            # ---- V projection ----
            for nb in (2 * mc, 2 * mc + 1):
                pv = psum.tile([128, 512], FP32, tag="mm", name="mm")
                for kc in range(4):
                    nc.tensor.matmul(
                        pv[:],
                        xt[kc][:, nb * 128 : (nb + 1) * 128],
                        w_t["wv", kc][:],
                        start=(kc == 0),
                        stop=(kc == 3),
                    )
                for h in range(NH):
                    nc.vector.tensor_copy(vaug[h][nb][:, 0:64], pv[:, h * 64 : h * 64 + 64])

        # ---- attention: S^T/exp of head h interleaved with AV of head h-1 ----
        est_all = [None] * NH

        def emit_st(h):
            est = [expp.tile([128, N], BF16, tag="expst", name="expst") for jb in range(8)]
            est_all[h] = est
            for jb in range(8):
                pst = psum.tile([128, N], FP32, tag="mm", name="mm")
                for nn in range(2):
                    nc.tensor.matmul(
                        pst[:, nn * 512 : (nn + 1) * 512],
                        kaug[h][:, jb * 128 : (jb + 1) * 128],
                        qaug[h][:, nn * 512 : (nn + 1) * 512],
                        start=True,
                        stop=True,
                    )
                nc.scalar.activation(est[jb][:], pst[:], Exp, bias=0.0, scale=0.125)

        def emit_av(h):
            est = est_all[h]
            pav = psum_av.tile([128, N], FP32, tag="av", name="av")
            for jb in range(8):
                for nn in range(2):
                    nc.tensor.matmul(
                        pav[:, nn * 512 : (nn + 1) * 512],
                        vaug[h][jb][:],
                        est[jb][:, nn * 512 : (nn + 1) * 512],
                        start=(jb == 0),
                        stop=(jb == 7),
                    )
            # NB: reciprocal_approx_fast (custom DVE op) breaks on partition-
            # offset sources — stage the denominator at partition base 0 first.
            # Keep the whole normalize chain on DVE so it hides under the
            # next head's (ACT-paced) S^T/exp stream.
            den = work.tile([64, N], FP32, tag="den", name="den", bufs=1)
            nc.vector.tensor_copy(den[:], pav[64:128, :])
            recip = work.tile([64, N], FP32, tag="recip", name="recip", bufs=1)
            nc.vector.reciprocal_approx_fast(recip[:], den[:])
            nc.vector.tensor_mul(
                oin[h // 2][(h % 2) * 64 : (h % 2) * 64 + 64, :], pav[0:64, :], recip[:]
            )

        emit_st(0)
        for h in range(1, NH):
            emit_st(h)
            emit_av(h - 1)
        emit_av(NH - 1)

        # ---------------- O projection + bias ----------------
        for mc in range(4):
            po = psum.tile([128, N], FP32, tag="mm", name="mm")
            for kc in range(4):
                for nn in range(2):
                    nc.tensor.matmul(
                        po[:, nn * 512 : (nn + 1) * 512],
                        w_t["wo", kc][:, mc * 128 : (mc + 1) * 128],
                        oin[kc][:, nn * 512 : (nn + 1) * 512],
                        start=(kc == 0),
                        stop=(kc == 3),
                    )
            oo = work.tile([128, N], FP32, tag="oout", name="oout")
            nc.vector.tensor_add(oo[:], po[:], bo_t[:, mc : mc + 1].broadcast_to((128, N)))
            nc.sync.dma_start(y_out[b, mc * 128 : (mc + 1) * 128, :], oo[:])


def _host_prep(w_q, w_k, w_v, w_o, b_o, rel_h, rel_w):
    import ml_dtypes

    perm = np.array([(c % 64) * 8 + c // 64 for c in range(C)])  # c' -> orig c
    oh = np.zeros((64, N), np.float32)
    j = np.arange(N)
    oh[j // HW, j] = 1.0
    oh[32 + j % HW, j] = 1.0
    # block-diag slices: col block qx holds [headA: rel_h.T[:, 31-qx:63-qx] | 0]
    # stacked with [0 | headB: same slice] so one K=128 M=64 matmul yields
    # both heads' ahT rows.
    rhT = rel_h.T.astype(np.float32)
    rwT = rel_w.T.astype(np.float32)
    rh2 = np.zeros((128, 2048), np.float32)
    rw2 = np.zeros((128, 2048), np.float32)
    for qq in range(32):
        sl = slice(31 - qq, 63 - qq)
        rh2[0:64, qq * 64 : qq * 64 + 32] = rhT[:, sl]
        rh2[64:128, qq * 64 + 32 : qq * 64 + 64] = rhT[:, sl]
        rw2[0:64, qq * 64 : qq * 64 + 32] = rwT[:, sl]
        rw2[64:128, qq * 64 + 32 : qq * 64 + 64] = rwT[:, sl]
    rh2 = rh2.astype(ml_dtypes.bfloat16)
    rw2 = rw2.astype(ml_dtypes.bfloat16)
    return dict(
        wq=np.ascontiguousarray(w_q[perm, :].T, dtype=np.float32),
        wk=np.ascontiguousarray(w_k[perm, :].T, dtype=np.float32),
        wv=np.ascontiguousarray(w_v[perm, :].T, dtype=np.float32),
        wo=np.ascontiguousarray(w_o.T, dtype=np.float32),
        onehot=oh,
        relh2=rh2,
        relw2=rw2,
        bo=np.ascontiguousarray(b_o.reshape(C, 1), dtype=np.float32),
    )


_CACHE = {}


def _build_program():
    if "nc" in _CACHE:
        return _CACHE["nc"], _CACHE["names"]
    nc = bacc.Bacc("TRN2", target_bir_lowering=False, debug=False, num_devices=NCORES)
    specs = [
        ("x", (NB, C, N), FP32R),
        ("wq", (C, C), FP32R),
        ("wk", (C, C), FP32R),
        ("wv", (C, C), FP32R),
        ("wo", (C, C), FP32R),
        ("onehot", (64, N), FP32R),
        ("relh2", (128, 2048), BF16),
        ("relw2", (128, 2048), BF16),
        ("bo", (C, 1), FP32),
    ]
    in_aps = [nc.dram_tensor(nm, list(shape), dt, kind="ExternalInput").ap() for nm, shape, dt in specs]
    out_ap = nc.dram_tensor("y", [NB, C, N], FP32, kind="ExternalOutput").ap()
    with tile.TileContext(nc) as tc:
        with ExitStack() as ctx:
            _build_body(ctx, tc, [out_ap], in_aps, NB)
    nc.compile()
    _CACHE["nc"] = nc
    _CACHE["names"] = [s[0] for s in specs]
    return nc, _CACHE["names"]


def _run(inputs, trace=False, tmpdir=None):
    x = np.asarray(inputs["x"], dtype=np.float32)
    cst = _host_prep(
        np.asarray(inputs["w_q"], np.float32),
        np.asarray(inputs["w_k"], np.float32),
        np.asarray(inputs["w_v"], np.float32),
        np.asarray(inputs["w_o"], np.float32),
        np.asarray(inputs["b_o"], np.float32),
        np.asarray(inputs["rel_h"], np.float32),
        np.asarray(inputs["rel_w"], np.float32),
    )
    nc, _ = _build_program()
    in_maps = []
    for c in range(NCORES):
        m = dict(cst)
        m["x"] = np.ascontiguousarray(x[c * NB : (c + 1) * NB].reshape(NB, C, N))
        in_maps.append(m)
    res = run_bass_kernel_spmd(
        nc, in_maps, core_ids=list(range(NCORES)), trace=trace, tmpdir=tmpdir
    )
    out = np.empty((B, C, HW, HW), np.float32)
    for c in range(NCORES):
        out[c * NB : (c + 1) * NB] = res.results[c]["y"].reshape(NB, C, HW, HW)
    return out, res


def kernel(**inputs):
    out, _ = _run(inputs, trace=False)
    return out
